# revision 1
# baseline (speedup 1.0000x reference)
"""Trainium2 Bass kernel for nn_Mk1_91036126806096.

Shared-weight LSTM (3 units, all-sigmoid activations) over [192 folded
sequences x T=4096 x 64 features], followed by a 4-unit dense layer with
sigmoid.  Data-parallel over 8 NeuronCores (8 original batch elements,
i.e. 24 folded sequences, per core).

The sequential scan is replaced by a Picard fixed-point iteration: given
gate values the c-recurrence c_t = f_t*c_{t-1} + i_t*g_t is linear and
runs in one DVE tensor_tensor_scan instruction per 512-step chunk; the
gates are recomputed from the lagged h trajectory each sweep.  The
iteration contracts by ~10x per sweep (verified vs the sequential
reference), so K sweeps reach the fp32 noise floor for K >= 8.

Per-core layout: "lane" L = 3*s + u for folded sequence s = 3*b + c
(b = local batch 0..7, c = feature chunk 0..2) and unit u.  Everything in
phase 2 lives on lanes 0..71 with time (and the 4 gates, as 4 blocks)
along the free dimension, so gate slicing is free-dim only.

Phase-2 matmuls run in float32r (single-pass PE, ~11-bit operand
rounding); phase 1 (column-packed, fp32r cannot column-tile) and the
final dense matmul stay fp32.  Emulated end-to-end error of this mix vs
the fp32 reference: ~1.8e-5 absolute on outputs in (0,1).
"""

import numpy as np

UNITS = 3
GATES = 4
B_FULL = 64
T_FULL = 4096
F = 64
N_CORES = 8
NB = 8                 # batch elements per core
NS = NB * 3            # folded sequences per core
L = NS * UNITS         # lanes = 72
TC = 512               # time chunk (one PSUM bank of fp32)
K_ITERS = 8            # Picard sweeps
MM_R = True            # float32r matmuls for phases 1-2

_cache = {}
TRACE = False
_last_exec_ns = None


def _build_module(T, k_iters, mm_r, debug):
    import concourse.bass as bass
    import concourse.tile as tile
    from concourse import bacc, mybir

    f32 = mybir.dt.float32
    mmdt = mybir.dt.float32r if mm_r else f32
    AF = mybir.ActivationFunctionType
    OP = mybir.AluOpType
    NCH = T // TC
    HT = T // 2

    nc = bacc.Bacc("TRN2", target_bir_lowering=False, debug=debug)

    xt = nc.dram_tensor("xt", [NS, F, T], f32, kind="ExternalInput")
    w_d = nc.dram_tensor("w", [2 * F, 12], f32, kind="ExternalInput")
    iz_d = nc.dram_tensor("iz", [L + 1, GATES * L], mmdt, kind="ExternalInput")
    bdu_d = nc.dram_tensor("bdu", [L, GATES * L], mmdt, kind="ExternalInput")
    s3_d = nc.dram_tensor("s3", [L, 4 * NB], f32, kind="ExternalInput")
    bdv_d = nc.dram_tensor("bdv", [4 * NB, 1], f32, kind="ExternalInput")
    ones_d = nc.dram_tensor("ones1", [1, GATES * T], mmdt, kind="ExternalInput")
    zeros_d = nc.dram_tensor("zeros1", [L, 1 + T], mmdt, kind="ExternalInput")
    y_d = nc.dram_tensor("y", [4 * NB, T], f32, kind="ExternalOutput")

    with tile.TileContext(nc) as tc:
        with tc.tile_pool(name="const", bufs=1) as cp, \
             tc.tile_pool(name="persist", bufs=1) as pp:
            w_t = cp.tile([2 * F, 12], f32, tag="w")
            nc.sync.dma_start(w_t[:], w_d.ap())
            iz_t = cp.tile([L + 1, GATES * L], mmdt, tag="iz")
            nc.sync.dma_start(iz_t[:], iz_d.ap())
            bdu_t = cp.tile([L, GATES * L], mmdt, tag="bdu")
            nc.sync.dma_start(bdu_t[:], bdu_d.ap())
            s3_t = cp.tile([L, 4 * NB], f32, tag="s3")
            nc.sync.dma_start(s3_t[:], s3_d.ap())
            bdv_t = cp.tile([4 * NB, 1], f32, tag="bdv")
            nc.sync.dma_start(bdv_t[:], bdv_d.ap())

            zpre = pp.tile([L + 1, GATES * T], mmdt, tag="zpre")
            nc.sync.dma_start(zpre[L:L + 1, :], ones_d.ap())
            hA = pp.tile([L, 1 + T], mmdt, tag="hA")
            hB = pp.tile([L, 1 + T], mmdt, tag="hB")
            nc.sync.dma_start(hA[:, :], zeros_d.ap())
            nc.sync.dma_start(hB[:, 0:1], zeros_d.ap()[:, 0:1])

            # ---------------- Phase 1: zpre = x @ W ----------------
            # 4 seqs per PSUM tile via column-group packing; staging
            # holds the whole T so the scatter to zpre's (s,u)-major
            # layout is 16 large DMAs per group of 4 seqs.
            with tc.tile_pool(name="xp", bufs=2) as xp, \
                 tc.tile_pool(name="stgp", bufs=2) as stgp, \
                 tc.tile_pool(name="ps1", bufs=1, space="PSUM") as ps1p:
                pts = []
                for i in range(3):
                    pt = ps1p.tile([128, TC], f32, tag=f"p1b{i}")
                    nc.vector.memset(pt[:, :], 0.0)
                    pts.append(pt)
                it = 0
                for g in range(NS // 4):
                    stg = stgp.tile([108, T], mmdt, tag="stg")
                    for half in range(2):
                        xA = xp.tile([128, HT], f32, tag="xA")
                        xB = xp.tile([128, HT], f32, tag="xB")
                        nc.sync.dma_start(
                            xA[:], xt.ap()[4 * g:4 * g + 2, :,
                                           half * HT:(half + 1) * HT])
                        nc.sync.dma_start(
                            xB[:], xt.ap()[4 * g + 2:4 * g + 4, :,
                                           half * HT:(half + 1) * HT])
                        for j in range(NCH // 2):
                            pt = pts[it % 3]
                            for q in range(4):
                                xtile = xA if q < 2 else xB
                                r0 = (q % 2) * 64
                                nc.tensor.matmul(
                                    pt[32 * q:32 * q + 12, :],
                                    w_t[r0:r0 + 64, :],
                                    xtile[r0:r0 + 64, j * TC:(j + 1) * TC],
                                    start=True, stop=True,
                                    tile_position=(r0, 32 * q))
                            col = (half * (NCH // 2) + j) * TC
                            if it % 2 == 0:
                                nc.scalar.copy(stg[:, col:col + TC],
                                               pt[0:108, :])
                            else:
                                nc.vector.tensor_copy(stg[:, col:col + TC],
                                                      pt[0:108, :])
                            it += 1
                    for q in range(4):
                        s = 4 * g + q
                        for gt in range(GATES):
                            eng = nc.sync if (q + gt) % 2 == 0 else nc.scalar
                            eng.dma_start(
                                zpre[3 * s:3 * s + 3, gt * T:(gt + 1) * T],
                                stg[32 * q + 3 * gt:32 * q + 3 * gt + 3, :])

            # ---------------- Phase 2: Picard sweeps ----------------
            with tc.tile_pool(name="sp", bufs=3) as sp, \
                 tc.tile_pool(name="igp", bufs=2) as igp, \
                 tc.tile_pool(name="scp", bufs=2) as scp, \
                 tc.tile_pool(name="cpool", bufs=3) as cpl, \
                 tc.tile_pool(name="zps", bufs=2, space="PSUM") as zpsp:
                hbufs = [hA, hB]
                for k in range(k_iters):
                    hold = hbufs[k % 2]
                    hnew = hbufs[(k + 1) % 2]
                    c_prev = None
                    for j in range(NCH):
                        zps = zpsp.tile([L, GATES * TC], f32, tag="zps")
                        for gt in range(GATES):
                            nc.tensor.matmul(
                                zps[:, gt * TC:(gt + 1) * TC],
                                iz_t[:, gt * L:(gt + 1) * L],
                                zpre[:, gt * T + j * TC:gt * T + (j + 1) * TC],
                                start=True, stop=False, tile_position=(0, 0))
                            nc.tensor.matmul(
                                zps[:, gt * TC:(gt + 1) * TC],
                                bdu_t[:, gt * L:(gt + 1) * L],
                                hold[:, j * TC:(j + 1) * TC],
                                start=False, stop=True, tile_position=(0, 0))
                        s_t = sp.tile([L, GATES * TC], f32, tag="s")
                        nc.scalar.activation(s_t[:], zps[:, :], AF.Sigmoid)
                        ig = igp.tile([L, TC], f32, tag="ig")
                        nc.vector.tensor_tensor(
                            out=ig[:], in0=s_t[:, 0:TC],
                            in1=s_t[:, 2 * TC:3 * TC], op=OP.mult)
                        c_t = cpl.tile([L, TC], f32, tag="c")
                        init = 0.0 if j == 0 else c_prev[:, TC - 1:TC]
                        nc.vector.tensor_tensor_scan(
                            out=c_t[:], data0=s_t[:, TC:2 * TC], data1=ig[:],
                            initial=init, op0=OP.mult, op1=OP.add)
                        c_prev = c_t
                        sc_t = scp.tile([L, TC], f32, tag="sc")
                        nc.scalar.activation(sc_t[:], c_t[:], AF.Sigmoid)
                        nc.vector.tensor_tensor(
                            out=hnew[:, 1 + j * TC:1 + (j + 1) * TC],
                            in0=s_t[:, 3 * TC:4 * TC], in1=sc_t[:], op=OP.mult)

            # ---------------- Phase 3: dense + sigmoid (fp32) -------
            hfin = hbufs[k_iters % 2]
            hfin_f = hfin[:].bitcast(f32) if mm_r else hfin[:]
            with tc.tile_pool(name="yp", bufs=2) as yp, \
                 tc.tile_pool(name="ps3", bufs=2, space="PSUM") as ps3p:
                for j in range(NCH):
                    p3 = ps3p.tile([4 * NB, TC], f32, tag="p3")
                    nc.tensor.matmul(
                        p3[:, :], s3_t[:, :],
                        hfin_f[:, 1 + j * TC:1 + (j + 1) * TC],
                        start=True, stop=True, tile_position=(0, 0))
                    y_t = yp.tile([4 * NB, TC], f32, tag="y")
                    nc.scalar.activation(y_t[:], p3[:, :], AF.Sigmoid,
                                         bias=bdv_t[:, :])
                    nc.sync.dma_start(y_d.ap()[:, j * TC:(j + 1) * TC], y_t[:])

    nc.compile()
    return nc


def _rnd11(v):
    """Round to 11 explicit mantissa bits (what fp32r keeps of operands)."""
    u = np.ascontiguousarray(v, np.float32).view(np.int32)
    s = 23 - 11
    return (((u + (1 << (s - 1))) >> s) << s).astype(np.int32).view(np.float32)


def _host_consts(W, U, b, Wd, bd, T, mm_r):
    """Pack the small parameter matrices into the stationary layouts."""
    W = np.asarray(W, np.float32)
    U = np.asarray(U, np.float32)
    b = np.asarray(b, np.float32)
    Wd = np.asarray(Wd, np.float32)
    bd = np.asarray(bd, np.float32)

    iz = np.zeros((L + 1, GATES * L), np.float32)
    bdu = np.zeros((L, GATES * L), np.float32)
    for gt in range(GATES):
        blk = iz[:, gt * L:(gt + 1) * L]
        blk[0:L, :] = np.eye(L, dtype=np.float32)
        for s in range(NS):
            for u in range(UNITS):
                blk[L, 3 * s + u] = b[3 * gt + u]
        ublk = bdu[:, gt * L:(gt + 1) * L]
        for s in range(NS):
            for up in range(UNITS):
                for u in range(UNITS):
                    ublk[3 * s + up, 3 * s + u] = U[up, 3 * gt + u]
    s3 = np.zeros((L, 4 * NB), np.float32)
    for bb in range(NB):
        for c in range(3):
            for u in range(UNITS):
                for d in range(4):
                    s3[9 * bb + 3 * c + u, 4 * bb + d] = Wd[3 * c + u, d]
    bdv = np.tile(bd, NB).reshape(4 * NB, 1).astype(np.float32)
    ones = np.ones((1, GATES * T), np.float32)
    zeros = np.zeros((L, 1 + T), np.float32)
    if mm_r:
        iz, bdu = _rnd11(iz), _rnd11(bdu)
    W2 = np.concatenate([W, W], axis=0)
    return {"w": W2, "iz": iz, "bdu": bdu, "s3": s3, "bdv": bdv, "ones1": ones,
            "zeros1": zeros}


def _host_xt(inputs, T):
    """[B, T, 192] -> per-core [NS, F, T] with s = 3*b_local + c."""
    B = inputs.shape[0]
    x = np.asarray(inputs, np.float32).reshape(B, T, 3, F)
    x = np.ascontiguousarray(np.transpose(x, (0, 2, 3, 1)))  # [B, c, F, T]
    per_core = []
    for k in range(N_CORES):
        per_core.append(x[k * NB:(k + 1) * NB].reshape(NS, F, T))
    return per_core


def kernel(inputs, W, U, b, Wd, bd):
    from concourse.bass_utils import run_bass_kernel_spmd

    B, T, F3 = inputs.shape
    assert (B, T, F3) == (B_FULL, T_FULL, 192)

    key = (T, K_ITERS, MM_R)
    if key not in _cache:
        _cache[key] = _build_module(T, K_ITERS, MM_R, debug=False)
    nc = _cache[key]

    consts = _host_consts(W, U, b, Wd, bd, T, MM_R)
    xts = _host_xt(inputs, T)
    in_maps = [dict(consts, xt=xts[k]) for k in range(N_CORES)]

    global _last_exec_ns
    res = run_bass_kernel_spmd(nc, in_maps, list(range(N_CORES)), trace=TRACE)
    if res.exec_time_ns is not None:
        _last_exec_ns = res.exec_time_ns
    ys = [res.results[k]["y"] for k in range(N_CORES)]  # [32, T] each

    out = np.empty((B, T, 4), np.float32)
    for k in range(N_CORES):
        blk = ys[k].reshape(NB, 4, T)          # [b, d, t]
        out[k * NB:(k + 1) * NB] = np.transpose(blk, (0, 2, 1))
    return out



# revision 7
# speedup vs baseline: 3.3006x; 3.3006x over previous
"""Trainium2 Bass kernel for nn_Mk1_91036126806096.

Shared-weight LSTM (3 units, all-sigmoid) over [192 folded seqs x T=4096
x 64 feat] + 4-unit sigmoid dense.  Data-parallel over 8 NeuronCores
(24 folded seqs per core).

The sequential scan is replaced by K=3 Picard sweeps: gates are
recomputed from the lagged h trajectory; the linear c-recurrence runs as
one DVE tensor_tensor_scan per 512-step chunk (fp32 scan state).  The
whole data path is bf16 (emulated end-to-end error vs the fp32
reference: ~2.7e-3 relative, tolerance 2e-2).

Layout: "su-lane" = 4*s + u for folded seq s = 3*b_local + c and unit
u (u=3 padded) -> 96 lanes.  Phase 1 streams x in seq-pairs [128 = 2x64
feat, T] through one shared stationary [128, 32] with 4-way PE column
tiling (4 pairs concurrently), giving PSUM tiles [128 = 4 pair-blocks x
(gate, member, unit), 512].  A copy folds the gate bias in (per-lane
bias vector) and casts to bf16; a one-time DMA regroup scatters the
(pair, gate)-strips into zpre96 [96 su-lanes, gate*T + t].

Sweep 1 needs no matmuls (h=0): sigmoid reads zpre96 directly.  Sweeps
2+ compute z = I@zpre + Ug@h_lag in PSUM (2 matmuls per gate per
512-chunk).  Rounds are software-pipelined so ACT's next-chunk gate
sigmoid is not queued behind the scan-dependent chain.
"""

import numpy as np
import ml_dtypes

UNITS = 3
GATES = 4
B_FULL = 64
T_FULL = 4096
F = 64
N_CORES = 8
NB = 8                 # batch elements per core
NS = NB * 3            # folded sequences per core = 24
NPAIR = NS // 2        # 12 seq-pairs per core
SU = 4 * NS            # su-lanes (u padded to 4) = 96
TC = 512               # time chunk (one PSUM bank of fp32)
NCH = T_FULL // TC     # 8
HT = T_FULL // 2       # 2048
K_ITERS = 3            # Picard sweeps

_cache = {}
TRACE = False
_last_exec_ns = None


def _build_module(T, k_iters, debug):
    import concourse.bass as bass
    import concourse.tile as tile
    from concourse import bacc, mybir

    f32 = mybir.dt.float32
    bf16 = mybir.dt.bfloat16
    AF = mybir.ActivationFunctionType
    OP = mybir.AluOpType

    nc = bacc.Bacc("TRN2", target_bir_lowering=False, debug=debug)

    xt_d = nc.dram_tensor("xt", [NPAIR, 128, T], bf16, kind="ExternalInput")
    s1_d = nc.dram_tensor("s1", [128, 32], bf16, kind="ExternalInput")
    bias1_d = nc.dram_tensor("bias1", [128, 1], f32, kind="ExternalInput")
    i96_d = nc.dram_tensor("i96", [SU, SU], bf16, kind="ExternalInput")
    u4_d = nc.dram_tensor("u4", [SU, GATES * SU], bf16, kind="ExternalInput")
    s3_d = nc.dram_tensor("s3", [SU, 4 * NB], bf16, kind="ExternalInput")
    bdv_d = nc.dram_tensor("bdv", [4 * NB, 1], f32, kind="ExternalInput")
    y_d = nc.dram_tensor("y", [4 * NB, T], f32, kind="ExternalOutput")

    # round-robin DMA issue over the two otherwise-idle queues
    qrr = {"i": 0}

    def next_q():
        qrr["i"] += 1
        return (nc.sync, nc.gpsimd)[qrr["i"] % 2]

    with tile.TileContext(nc) as tc:
        with tc.tile_pool(name="const", bufs=1) as cp, \
             tc.tile_pool(name="persist", bufs=1) as pp, \
             tc.tile_pool(name="sp", bufs=3) as sp, \
             tc.tile_pool(name="igp", bufs=2) as igp, \
             tc.tile_pool(name="cpool", bufs=2) as cpl, \
             tc.tile_pool(name="scp", bufs=2) as scp:
            s1_t = cp.tile([128, 32], bf16, tag="s1")
            nc.sync.dma_start(s1_t[:], s1_d.ap())
            bias1_t = cp.tile([128, 1], f32, tag="bias1")
            nc.sync.dma_start(bias1_t[:], bias1_d.ap())
            i96_t = cp.tile([SU, SU], bf16, tag="i96")
            nc.sync.dma_start(i96_t[:], i96_d.ap())
            u4_t = cp.tile([SU, GATES * SU], bf16, tag="u4")
            nc.sync.dma_start(u4_t[:], u4_d.ap())
            s3_t = cp.tile([SU, 4 * NB], bf16, tag="s3")
            nc.sync.dma_start(s3_t[:], s3_d.ap())
            bdv_t = cp.tile([4 * NB, 1], f32, tag="bdv")
            nc.sync.dma_start(bdv_t[:], bdv_d.ap())

            zpre96 = pp.tile([SU, GATES * T], bf16, tag="zpre96")
            zpre3 = [pp.tile([128, T], bf16, tag=f"z3_{t3}", name=f"zpre3_{t3}")
                     for t3 in range(3)]
            hA = pp.tile([SU, 1 + T], bf16, tag="hA")
            hB = pp.tile([SU, 1 + T], bf16, tag="hB")
            nc.vector.memset(hA[:, 0:1], 0.0)
            nc.vector.memset(hB[:, 0:1], 0.0)
            hbufs = [hA, hB]

            # ---------- phase-2 pipelined round ----------
            st = {}

            def sweep_round(k, j, hold, hnew, zpsp):
                if j < NCH:
                    s_t = sp.tile([SU, GATES * TC], bf16, tag="s")
                    if k == 0:
                        src = zpre96[:, :].rearrange(
                            "p (g t) -> p g t", g=GATES)[:, :, j * TC:(j + 1) * TC]
                        dst = s_t[:, :].rearrange("p (g t) -> p g t", g=GATES)
                        nc.scalar.activation(dst, src, AF.Sigmoid)
                    else:
                        zps = zpsp.tile([SU, GATES * TC], f32, tag="zps")
                        for g in range(GATES):
                            nc.tensor.matmul(
                                zps[:, g * TC:(g + 1) * TC], i96_t[:, :],
                                zpre96[:, g * T + j * TC:g * T + (j + 1) * TC],
                                start=True, stop=False)
                        for g in range(GATES):
                            nc.tensor.matmul(
                                zps[:, g * TC:(g + 1) * TC],
                                u4_t[:, SU * g:SU * (g + 1)],
                                hold[:, j * TC:(j + 1) * TC],
                                start=False, stop=True)
                        nc.scalar.activation(s_t[:, :], zps[:, :], AF.Sigmoid)
                    st[("s", j)] = s_t
                jj = j - 1
                if 0 <= jj:
                    s_t = st.pop(("s", jj))
                    ig = igp.tile([SU, TC], bf16, tag="ig")
                    nc.vector.tensor_tensor(
                        out=ig[:, :], in0=s_t[:, 0:TC],
                        in1=s_t[:, 2 * TC:3 * TC], op=OP.mult)
                    c_t = cpl.tile([SU, TC], bf16, tag="c")
                    init = 0.0 if jj == 0 else st["c_prev"][:, TC - 1:TC]
                    nc.vector.tensor_tensor_scan(
                        out=c_t[:, :], data0=s_t[:, TC:2 * TC], data1=ig[:, :],
                        initial=init, op0=OP.mult, op1=OP.add)
                    st["c_prev"] = c_t
                    sc = scp.tile([SU, TC], bf16, tag="sc")
                    nc.scalar.activation(sc[:, :], c_t[:, :], AF.Sigmoid)
                    nc.vector.tensor_tensor(
                        out=hnew[:, 1 + jj * TC:1 + (jj + 1) * TC],
                        in0=s_t[:, 3 * TC:4 * TC], in1=sc[:, :], op=OP.mult)

            # ---------- phase 1 ----------
            with tc.tile_pool(name="xp", bufs=2) as xp, \
                 tc.tile_pool(name="ps1", bufs=2, space="PSUM") as ps1p:
                xtiles = {}

                def dma_half(half):
                    for tau in range(3):
                        for p in range(4):
                            xt_t = xp.tile([128, HT], bf16, tag=f"xp{tau}{p}")
                            next_q().dma_start(
                                xt_t[:],
                                xt_d.ap()[4 * tau + p, :,
                                          half * HT:(half + 1) * HT])
                            xtiles[(tau, p)] = xt_t

                def compute_half(half):
                    cpy = half * 12
                    for tau in range(3):
                        for jc in range(4):
                            pt = ps1p.tile([128, TC], f32, tag="p1")
                            for p in range(4):
                                nc.tensor.matmul(
                                    pt[32 * p:32 * p + 32, :], s1_t[:, :],
                                    xtiles[(tau, p)][:, jc * TC:(jc + 1) * TC],
                                    start=True, stop=True,
                                    tile_position=(0, 32 * p))
                            dcol = half * HT + jc * TC
                            dst = zpre3[tau][:, dcol:dcol + TC]
                            if cpy % 2 == 0:
                                nc.scalar.activation(dst, pt[:, :], AF.Identity,
                                                     bias=bias1_t[:, :])
                            else:
                                nc.vector.tensor_scalar(
                                    dst, pt[:, :], bias1_t[:, :], None,
                                    op0=OP.add)
                            cpy += 1

                def regroup_half(half):
                    # zpre3 lanes within a 32-block are (m, u, g) = 16m+4u+g,
                    # so a contiguous [32, HT] source streams elementwise as
                    # ((m,u), g, t) -- exactly the dst [8 parts, 4 g-blocks,
                    # HT] iteration order.
                    for tau in range(3):
                        for q in range(4):
                            src = zpre3[tau][32 * q:32 * q + 32,
                                             half * HT:(half + 1) * HT]
                            dst = zpre96[32 * tau + 8 * q:32 * tau + 8 * q + 8, :] \
                                .rearrange("p (g t) -> p g t", g=GATES) \
                                [:, :, half * HT:(half + 1) * HT]
                            next_q().dma_start(dst, src)

                dma_half(0)
                compute_half(0)
                regroup_half(0)
                dma_half(1)
                # sweep-1 first half overlaps phase-1 second half
                for j in range(NCH // 2):
                    sweep_round(0, j, hbufs[0], hbufs[1], None)
                compute_half(1)
                regroup_half(1)

            with tc.tile_pool(name="zps", bufs=2, space="PSUM") as zpsp:
                for j in range(NCH // 2, NCH + 1):
                    sweep_round(0, j, hbufs[0], hbufs[1], zpsp)
                for k in range(1, k_iters):
                    hold = hbufs[k % 2]
                    hnew = hbufs[(k + 1) % 2]
                    for j in range(NCH + 1):
                        sweep_round(k, j, hold, hnew, zpsp)

            # ---------- phase 3: dense + sigmoid ----------
            hfin = hbufs[k_iters % 2]
            with tc.tile_pool(name="yp", bufs=2) as yp, \
                 tc.tile_pool(name="ps3", bufs=2, space="PSUM") as ps3p:
                for j in range(NCH):
                    p3 = ps3p.tile([4 * NB, TC], f32, tag="p3")
                    nc.tensor.matmul(
                        p3[:, :], s3_t[:, :],
                        hfin[:, 1 + j * TC:1 + (j + 1) * TC],
                        start=True, stop=True)
                    y_t = yp.tile([4 * NB, TC], f32, tag="y")
                    nc.scalar.activation(y_t[:], p3[:, :], AF.Sigmoid,
                                         bias=bdv_t[:, :])
                    next_q().dma_start(y_d.ap()[:, j * TC:(j + 1) * TC], y_t[:])

    nc.compile()
    return nc


def _host_consts(W, U, b, Wd, bd):
    W = np.asarray(W, np.float32)
    U = np.asarray(U, np.float32)
    b = np.asarray(b, np.float32)
    Wd = np.asarray(Wd, np.float32)
    bd = np.asarray(bd, np.float32)
    bf = ml_dtypes.bfloat16

    # phase-1 psum within-block col order is (m, u, g) = 16m + 4u + g
    s1 = np.zeros((128, 32), np.float32)
    for m in range(2):
        for g in range(GATES):
            for u in range(UNITS):
                s1[64 * m:64 * m + 64, 16 * m + 4 * u + g] = W[:, 3 * g + u]

    # bias per natural lane: lane 32q + 16m + 4u + g -> b[3g+u]
    blk = np.zeros(32, np.float32)
    for g in range(GATES):
        for m in range(2):
            for u in range(UNITS):
                blk[16 * m + 4 * u + g] = b[3 * g + u]
    bias1 = np.tile(blk, 4).reshape(128, 1).astype(np.float32)

    i96 = np.eye(SU, dtype=np.float32)

    u4 = np.zeros((SU, GATES * SU), np.float32)
    for s in range(NS):
        for up in range(UNITS):
            for g in range(GATES):
                for u in range(UNITS):
                    u4[4 * s + up, SU * g + 4 * s + u] = U[up, 3 * g + u]

    s3 = np.zeros((SU, 4 * NB), np.float32)
    for b_ in range(NB):
        for c in range(3):
            s = 3 * b_ + c
            for u in range(UNITS):
                for dd in range(4):
                    s3[4 * s + u, 4 * b_ + dd] = Wd[3 * c + u, dd]
    bdv = np.tile(bd, NB).reshape(4 * NB, 1).astype(np.float32)

    return {"s1": s1.astype(bf), "bias1": bias1, "i96": i96.astype(bf),
            "u4": u4.astype(bf), "s3": s3.astype(bf), "bdv": bdv}


def _host_xt(inputs, T):
    """[B, T, 192] -> per-core [NPAIR, 128, T] bf16, seq s = 3*b_local + c."""
    B = inputs.shape[0]
    x = np.asarray(inputs, np.float32).reshape(B, T, 3, F)
    x = np.transpose(x, (0, 2, 3, 1))              # [B, c, F, T]
    x = np.ascontiguousarray(x).astype(ml_dtypes.bfloat16)
    per_core = []
    for k in range(N_CORES):
        xc = x[k * NB:(k + 1) * NB].reshape(NS, F, T)   # s = 3b+c
        per_core.append(np.ascontiguousarray(xc.reshape(NPAIR, 128, T)))
    return per_core


def kernel(inputs, W, U, b, Wd, bd):
    from concourse.bass_utils import run_bass_kernel_spmd

    B, T, F3 = inputs.shape
    assert (B, T, F3) == (B_FULL, T_FULL, 192)

    key = (T, K_ITERS)
    if key not in _cache:
        _cache[key] = _build_module(T, K_ITERS, debug=False)
    nc = _cache[key]

    consts = _host_consts(W, U, b, Wd, bd)
    xts = _host_xt(inputs, T)
    in_maps = [dict(consts, xt=xts[k]) for k in range(N_CORES)]

    global _last_exec_ns
    res = run_bass_kernel_spmd(nc, in_maps, list(range(N_CORES)), trace=TRACE)
    if res.exec_time_ns is not None:
        _last_exec_ns = res.exec_time_ns
    ys = [res.results[k]["y"] for k in range(N_CORES)]  # [32, T] each

    out = np.empty((B, T, 4), np.float32)
    for k in range(N_CORES):
        blk = ys[k].reshape(NB, 4, T)          # [b, d, t]
        out[k * NB:(k + 1) * NB] = np.transpose(blk, (0, 2, 1))
    return out


# revision 12
# speedup vs baseline: 3.9869x; 1.2080x over previous
"""Trainium2 Bass kernel for nn_Mk1_91036126806096.

Shared-weight LSTM (3 units, all-sigmoid) over [192 folded seqs x T=4096
x 64 feat] + 4-unit sigmoid dense.  Data-parallel over 8 NeuronCores
(24 folded seqs per core).

The sequential scan is replaced by K=3 Picard sweeps: gates are
recomputed from the lagged h trajectory; the linear c-recurrence runs as
one DVE tensor_tensor_scan per 512-step chunk (fp32 scan state).  The
whole data path is bf16 (emulated end-to-end error vs the fp32
reference: ~2.7e-3 relative, tolerance 2e-2).

Layout: "su-lane" = 4*s + u for folded seq s = 3*b_local + c and unit
u (u=3 padded) -> 96 lanes.  Phase 1 streams x in seq-pairs [128 = 2x64
feat, T] through one shared stationary [128, 32] with 4-way PE column
tiling (4 pairs concurrently), giving PSUM tiles [128 = 4 pair-blocks x
(gate, member, unit), 512].  A copy folds the gate bias in (per-lane
bias vector) and casts to bf16; a one-time DMA regroup scatters the
(pair, gate)-strips into zpre96 [96 su-lanes, gate*T + t].

Sweep 1 needs no matmuls (h=0): sigmoid reads zpre96 directly.  Sweeps
2+ compute z = I@zpre + Ug@h_lag in PSUM (2 matmuls per gate per
512-chunk).  Rounds are software-pipelined so ACT's next-chunk gate
sigmoid is not queued behind the scan-dependent chain.
"""

import numpy as np
import ml_dtypes

UNITS = 3
GATES = 4
B_FULL = 64
T_FULL = 4096
F = 64
N_CORES = 8
NB = 8                 # batch elements per core
NS = NB * 3            # folded sequences per core = 24
NPAIR = NS // 2        # 12 seq-pairs per core
SU = 4 * NS            # su-lanes (u padded to 4) = 96
TC = 512               # time chunk (one PSUM bank of fp32)
NCH = T_FULL // TC     # 8
HT = T_FULL // 2       # 2048
K_ITERS = 3            # Picard sweeps

_cache = {}
TRACE = False
_last_exec_ns = None


def _build_module(T, k_iters, debug):
    import concourse.bass as bass
    import concourse.tile as tile
    from concourse import bacc, mybir

    f32 = mybir.dt.float32
    bf16 = mybir.dt.bfloat16
    AF = mybir.ActivationFunctionType
    OP = mybir.AluOpType

    nc = bacc.Bacc("TRN2", target_bir_lowering=False, debug=debug)

    xt_d = nc.dram_tensor("xt", [128, NPAIR * T], bf16, kind="ExternalInput")
    s1_d = nc.dram_tensor("s1", [128, 32], bf16, kind="ExternalInput")
    bias1_d = nc.dram_tensor("bias1", [128, 1], f32, kind="ExternalInput")
    i96_d = nc.dram_tensor("i96", [SU, SU], bf16, kind="ExternalInput")
    u4_d = nc.dram_tensor("u4", [SU, GATES * SU], bf16, kind="ExternalInput")
    s3_d = nc.dram_tensor("s3", [SU, 4 * NB], bf16, kind="ExternalInput")
    bdv_d = nc.dram_tensor("bdv", [4 * NB, 1], f32, kind="ExternalInput")
    y_d = nc.dram_tensor("y", [4 * NB, T], f32, kind="ExternalOutput")

    # round-robin DMA issue over the two otherwise-idle queues
    qrr = {"i": 0}

    def next_q():
        qrr["i"] += 1
        return (nc.sync, nc.gpsimd)[qrr["i"] % 2]

    with tile.TileContext(nc) as tc:
        with tc.tile_pool(name="const", bufs=1) as cp, \
             tc.tile_pool(name="persist", bufs=1) as pp, \
             tc.tile_pool(name="sp", bufs=3) as sp, \
             tc.tile_pool(name="igp", bufs=2) as igp, \
             tc.tile_pool(name="cpool", bufs=2) as cpl, \
             tc.tile_pool(name="scp", bufs=2) as scp:
            s1_t = cp.tile([128, 32], bf16, tag="s1")
            nc.sync.dma_start(s1_t[:], s1_d.ap())
            bias1_t = cp.tile([128, 1], f32, tag="bias1")
            nc.sync.dma_start(bias1_t[:], bias1_d.ap())
            i96_t = cp.tile([SU, SU], bf16, tag="i96")
            nc.sync.dma_start(i96_t[:], i96_d.ap())
            u4_t = cp.tile([SU, GATES * SU], bf16, tag="u4")
            nc.sync.dma_start(u4_t[:], u4_d.ap())
            s3_t = cp.tile([SU, 4 * NB], bf16, tag="s3")
            nc.sync.dma_start(s3_t[:], s3_d.ap())
            bdv_t = cp.tile([4 * NB, 1], f32, tag="bdv")
            nc.sync.dma_start(bdv_t[:], bdv_d.ap())

            zpre96 = pp.tile([SU, GATES * T], bf16, tag="zpre96")
            zpre3 = [pp.tile([128, T], bf16, tag=f"z3_{t3}", name=f"zpre3_{t3}")
                     for t3 in range(3)]
            hA = pp.tile([SU, 1 + T], bf16, tag="hA")
            hB = pp.tile([SU, 1 + T], bf16, tag="hB")
            nc.vector.memset(hA[:, 0:1], 0.0)
            nc.vector.memset(hB[:, 0:1], 0.0)
            hbufs = [hA, hB]

            # ---------- phase-2 pipelined round ----------
            st = {}

            def sweep_round(k, j, hold, hnew, zpsp):
                if j < NCH:
                    s_t = sp.tile([SU, GATES * TC], bf16, tag="s")
                    if k == 0:
                        src = zpre96[:, :].rearrange(
                            "p (g t) -> p g t", g=GATES)[:, :, j * TC:(j + 1) * TC]
                        dst = s_t[:, :].rearrange("p (g t) -> p g t", g=GATES)
                        nc.scalar.activation(dst, src, AF.Sigmoid)
                    else:
                        zps = zpsp.tile([SU, GATES * TC], f32, tag="zps")
                        for g in range(GATES):
                            nc.tensor.matmul(
                                zps[:, g * TC:(g + 1) * TC], i96_t[:, :],
                                zpre96[:, g * T + j * TC:g * T + (j + 1) * TC],
                                start=True, stop=False)
                        for g in range(GATES):
                            nc.tensor.matmul(
                                zps[:, g * TC:(g + 1) * TC],
                                u4_t[:, SU * g:SU * (g + 1)],
                                hold[:, j * TC:(j + 1) * TC],
                                start=False, stop=True)
                        nc.scalar.activation(s_t[:, :], zps[:, :], AF.Sigmoid)
                    st[("s", j)] = s_t
                jj = j - 1
                if 0 <= jj:
                    s_t = st.pop(("s", jj))
                    ig = igp.tile([SU, TC], bf16, tag="ig")
                    nc.vector.tensor_tensor(
                        out=ig[:, :], in0=s_t[:, 0:TC],
                        in1=s_t[:, 2 * TC:3 * TC], op=OP.mult)
                    c_t = cpl.tile([SU, TC], bf16, tag="c")
                    init = 0.0 if jj == 0 else st["c_prev"][:, TC - 1:TC]
                    nc.vector.tensor_tensor_scan(
                        out=c_t[:, :], data0=s_t[:, TC:2 * TC], data1=ig[:, :],
                        initial=init, op0=OP.mult, op1=OP.add)
                    st["c_prev"] = c_t
                    sc = scp.tile([SU, TC], bf16, tag="sc")
                    nc.scalar.activation(sc[:, :], c_t[:, :], AF.Sigmoid)
                    nc.vector.tensor_tensor(
                        out=hnew[:, 1 + jj * TC:1 + (jj + 1) * TC],
                        in0=s_t[:, 3 * TC:4 * TC], in1=sc[:, :], op=OP.mult)

            # ---------- phase 1 ----------
            with tc.tile_pool(name="xp", bufs=1) as xp:
                xbig = xp.tile([128, NPAIR * T], bf16, tag="xbig")
                # 4 large DMAs, alternating queues; 24KB contiguous per
                # partition row per transfer
                for i in range(4):
                    seg = NPAIR * T // 4
                    next_q().dma_start(xbig[:, i * seg:(i + 1) * seg],
                                       xt_d.ap()[:, i * seg:(i + 1) * seg])

                def warm(n, wt):
                    # PE warm-keeping: dummy matmuls into a scratch PSUM
                    # region (never read) so HAM doesn't re-throttle.
                    for i in range(n):
                        nc.tensor.matmul(wt[0:32, 0:TC], s1_t[:, :],
                                         xbig[:, (i % 8) * TC:
                                              (i % 8) * TC + TC],
                                         start=True, stop=True)

                with tc.tile_pool(name="ps1", bufs=2, space="PSUM") as ps1p:
                    def compute_half(half):
                        cpy = half * 12
                        for tau in range(3):
                            for jc in range(4):
                                pt = ps1p.tile([128, TC], f32, tag="p1")
                                for p in range(4):
                                    # x layout: [128, (half, pair, HT)]
                                    col = (half * NPAIR * HT
                                           + (4 * tau + p) * HT + jc * TC)
                                    nc.tensor.matmul(
                                        pt[32 * p:32 * p + 32, :], s1_t[:, :],
                                        xbig[:, col:col + TC],
                                        start=True, stop=True,
                                        tile_position=(0, 32 * p))
                                dcol = half * HT + jc * TC
                                dst = zpre3[tau][:, dcol:dcol + TC]
                                if cpy % 2 == 0:
                                    nc.scalar.activation(dst, pt[:, :],
                                                         AF.Identity,
                                                         bias=bias1_t[:, :])
                                else:
                                    nc.vector.tensor_scalar(
                                        dst, pt[:, :], bias1_t[:, :], None,
                                        op0=OP.add)
                                cpy += 1

                    def regroup_half(half):
                        # zpre3 lanes within a 32-block are (m, u, g) =
                        # 16m+4u+g, so a contiguous [32, HT] source streams
                        # elementwise as ((m,u), g, t) -- exactly the dst
                        # [8 parts, 4 g-blocks, HT] iteration order.
                        for tau in range(3):
                            for q in range(4):
                                src = zpre3[tau][32 * q:32 * q + 32,
                                                 half * HT:(half + 1) * HT]
                                dst = zpre96[32 * tau + 8 * q:
                                             32 * tau + 8 * q + 8, :] \
                                    .rearrange("p (g t) -> p g t", g=GATES) \
                                    [:, :, half * HT:(half + 1) * HT]
                                next_q().dma_start(dst, src)

                    compute_half(0)
                    regroup_half(0)
                    compute_half(1)
                    regroup_half(1)
                    # sweep-1 first half overlaps the regroup tail
                    for j in range(NCH // 2):
                        sweep_round(0, j, hbufs[0], hbufs[1], None)
                        wt = ps1p.tile([128, TC], f32, tag="wup")
                        warm(8, wt)

                with tc.tile_pool(name="zps", bufs=2, space="PSUM") as zpsp:
                    for j in range(NCH // 2, NCH + 1):
                        sweep_round(0, j, hbufs[0], hbufs[1], zpsp)
                        wt = zpsp.tile([SU, GATES * TC], f32, tag="zps")
                        warm(8, wt)
                    for k in range(1, k_iters):
                        hold = hbufs[k % 2]
                        hnew = hbufs[(k + 1) % 2]
                        for j in range(NCH + 1):
                            sweep_round(k, j, hold, hnew, zpsp)

            # ---------- phase 3: dense + sigmoid ----------
            hfin = hbufs[k_iters % 2]
            with tc.tile_pool(name="yp", bufs=2) as yp, \
                 tc.tile_pool(name="ps3", bufs=2, space="PSUM") as ps3p:
                for j in range(NCH):
                    p3 = ps3p.tile([4 * NB, TC], f32, tag="p3")
                    nc.tensor.matmul(
                        p3[:, :], s3_t[:, :],
                        hfin[:, 1 + j * TC:1 + (j + 1) * TC],
                        start=True, stop=True)
                    y_t = yp.tile([4 * NB, TC], f32, tag="y")
                    nc.scalar.activation(y_t[:], p3[:, :], AF.Sigmoid,
                                         bias=bdv_t[:, :])
                    next_q().dma_start(y_d.ap()[:, j * TC:(j + 1) * TC], y_t[:])

    nc.compile()
    return nc


def _host_consts(W, U, b, Wd, bd):
    W = np.asarray(W, np.float32)
    U = np.asarray(U, np.float32)
    b = np.asarray(b, np.float32)
    Wd = np.asarray(Wd, np.float32)
    bd = np.asarray(bd, np.float32)
    bf = ml_dtypes.bfloat16

    # phase-1 psum within-block col order is (m, u, g) = 16m + 4u + g
    s1 = np.zeros((128, 32), np.float32)
    for m in range(2):
        for g in range(GATES):
            for u in range(UNITS):
                s1[64 * m:64 * m + 64, 16 * m + 4 * u + g] = W[:, 3 * g + u]

    # bias per natural lane: lane 32q + 16m + 4u + g -> b[3g+u]
    blk = np.zeros(32, np.float32)
    for g in range(GATES):
        for m in range(2):
            for u in range(UNITS):
                blk[16 * m + 4 * u + g] = b[3 * g + u]
    bias1 = np.tile(blk, 4).reshape(128, 1).astype(np.float32)

    i96 = np.eye(SU, dtype=np.float32)

    u4 = np.zeros((SU, GATES * SU), np.float32)
    for s in range(NS):
        for up in range(UNITS):
            for g in range(GATES):
                for u in range(UNITS):
                    u4[4 * s + up, SU * g + 4 * s + u] = U[up, 3 * g + u]

    s3 = np.zeros((SU, 4 * NB), np.float32)
    for b_ in range(NB):
        for c in range(3):
            s = 3 * b_ + c
            for u in range(UNITS):
                for dd in range(4):
                    s3[4 * s + u, 4 * b_ + dd] = Wd[3 * c + u, dd]
    bdv = np.tile(bd, NB).reshape(4 * NB, 1).astype(np.float32)

    return {"s1": s1.astype(bf), "bias1": bias1, "i96": i96.astype(bf),
            "u4": u4.astype(bf), "s3": s3.astype(bf), "bdv": bdv}


def _host_xt(inputs, T):
    """[B, T, 192] -> per-core [128, (half, pair, HT)] bf16, s = 3*b_local+c."""
    B = inputs.shape[0]
    x = np.asarray(inputs, np.float32).reshape(B, T, 3, F)
    x = np.transpose(x, (0, 2, 3, 1))              # [B, c, F, T]
    x = np.ascontiguousarray(x).astype(ml_dtypes.bfloat16)
    per_core = []
    for k in range(N_CORES):
        xc = x[k * NB:(k + 1) * NB].reshape(NS, F, T)   # s = 3b+c
        xp = xc.reshape(NPAIR, 128, 2, HT)              # rows 64m+f
        xp = np.transpose(xp, (1, 2, 0, 3))             # [128, half, pair, HT]
        per_core.append(np.ascontiguousarray(xp).reshape(128, NPAIR * T))
    return per_core


def kernel(inputs, W, U, b, Wd, bd):
    from concourse.bass_utils import run_bass_kernel_spmd

    B, T, F3 = inputs.shape
    assert (B, T, F3) == (B_FULL, T_FULL, 192)

    key = (T, K_ITERS)
    if key not in _cache:
        _cache[key] = _build_module(T, K_ITERS, debug=False)
    nc = _cache[key]

    consts = _host_consts(W, U, b, Wd, bd)
    xts = _host_xt(inputs, T)
    in_maps = [dict(consts, xt=xts[k]) for k in range(N_CORES)]

    global _last_exec_ns
    res = run_bass_kernel_spmd(nc, in_maps, list(range(N_CORES)), trace=TRACE)
    if res.exec_time_ns is not None:
        _last_exec_ns = res.exec_time_ns
    ys = [res.results[k]["y"] for k in range(N_CORES)]  # [32, T] each

    out = np.empty((B, T, 4), np.float32)
    for k in range(N_CORES):
        blk = ys[k].reshape(NB, 4, T)          # [b, d, t]
        out[k * NB:(k + 1) * NB] = np.transpose(blk, (0, 2, 1))
    return out


# revision 15
# speedup vs baseline: 4.2966x; 1.0777x over previous
"""Trainium2 Bass kernel for nn_Mk1_91036126806096.

Shared-weight LSTM (3 units, all-sigmoid) over [192 folded seqs x T=4096
x 64 feat] + 4-unit sigmoid dense.  Data-parallel over 8 NeuronCores
(24 folded seqs per core).

The sequential scan is replaced by K=3 Picard sweeps: gates are
recomputed from the lagged h trajectory; the linear c-recurrence runs as
one DVE tensor_tensor_scan per 512-step chunk (fp32 scan state).  The
whole data path is bf16 (emulated end-to-end error vs the fp32
reference: ~2.7e-3 relative, tolerance 2e-2).

Layout: "su-lane" = 4*s + u for folded seq s = 3*b_local + c and unit
u (u=3 padded) -> 96 lanes.  Phase 1 streams x in seq-pairs [128 = 2x64
feat, T] through one shared stationary [128, 32] with 4-way PE column
tiling (4 pairs concurrently), giving PSUM tiles [128 = 4 pair-blocks x
(gate, member, unit), 512].  A copy folds the gate bias in (per-lane
bias vector) and casts to bf16; a one-time DMA regroup scatters the
(pair, gate)-strips into zpre96 [96 su-lanes, gate*T + t].

Sweep 1 needs no matmuls (h=0): sigmoid reads zpre96 directly.  Sweeps
2+ compute z = I@zpre + Ug@h_lag in PSUM (2 matmuls per gate per
512-chunk).  Rounds are software-pipelined so ACT's next-chunk gate
sigmoid is not queued behind the scan-dependent chain.
"""

import numpy as np
import ml_dtypes

UNITS = 3
GATES = 4
B_FULL = 64
T_FULL = 4096
F = 64
N_CORES = 8
NB = 8                 # batch elements per core
NS = NB * 3            # folded sequences per core = 24
NPAIR = NS // 2        # 12 seq-pairs per core
SU = 4 * NS            # su-lanes (u padded to 4) = 96
TC = 512               # time chunk (one PSUM bank of fp32)
NCH = T_FULL // TC     # 8
HT = T_FULL // 2       # 2048
K_ITERS = 2            # Picard sweeps

_cache = {}
TRACE = False
_last_exec_ns = None


def _build_module(T, k_iters, debug):
    import concourse.bass as bass
    import concourse.tile as tile
    from concourse import bacc, mybir

    f32 = mybir.dt.float32
    bf16 = mybir.dt.bfloat16
    AF = mybir.ActivationFunctionType
    OP = mybir.AluOpType

    nc = bacc.Bacc("TRN2", target_bir_lowering=False, debug=debug)

    xt_d = nc.dram_tensor("xt", [128, NPAIR * T], bf16, kind="ExternalInput")
    s1_d = nc.dram_tensor("s1", [128, 32], bf16, kind="ExternalInput")
    bias1_d = nc.dram_tensor("bias1", [128, 1], f32, kind="ExternalInput")
    i96_d = nc.dram_tensor("i96", [SU, SU], bf16, kind="ExternalInput")
    u4_d = nc.dram_tensor("u4", [SU, GATES * SU], bf16, kind="ExternalInput")
    s3_d = nc.dram_tensor("s3", [SU, 4 * NB], bf16, kind="ExternalInput")
    bdv_d = nc.dram_tensor("bdv", [4 * NB, 1], f32, kind="ExternalInput")
    y_d = nc.dram_tensor("y", [4 * NB, T], f32, kind="ExternalOutput")

    # round-robin DMA issue over the two otherwise-idle queues
    qrr = {"i": 0}

    def next_q():
        qrr["i"] += 1
        return (nc.sync, nc.gpsimd)[qrr["i"] % 2]

    with tile.TileContext(nc) as tc:
        with tc.tile_pool(name="const", bufs=1) as cp, \
             tc.tile_pool(name="persist", bufs=1) as pp, \
             tc.tile_pool(name="sp", bufs=3) as sp, \
             tc.tile_pool(name="igp", bufs=2) as igp, \
             tc.tile_pool(name="cpool", bufs=2) as cpl, \
             tc.tile_pool(name="scp", bufs=2) as scp:
            s1_t = cp.tile([128, 32], bf16, tag="s1")
            nc.sync.dma_start(s1_t[:], s1_d.ap())
            bias1_t = cp.tile([128, 1], f32, tag="bias1")
            nc.sync.dma_start(bias1_t[:], bias1_d.ap())
            i96_t = cp.tile([SU, SU], bf16, tag="i96")
            nc.sync.dma_start(i96_t[:], i96_d.ap())
            u4_t = cp.tile([SU, GATES * SU], bf16, tag="u4")
            nc.sync.dma_start(u4_t[:], u4_d.ap())
            s3_t = cp.tile([SU, 4 * NB], bf16, tag="s3")
            nc.sync.dma_start(s3_t[:], s3_d.ap())
            bdv_t = cp.tile([4 * NB, 1], f32, tag="bdv")
            nc.sync.dma_start(bdv_t[:], bdv_d.ap())

            zpre96 = pp.tile([SU, GATES * T], bf16, tag="zpre96")
            zpre3 = [pp.tile([128, T], bf16, tag=f"z3_{t3}", name=f"zpre3_{t3}")
                     for t3 in range(3)]
            hA = pp.tile([SU, 1 + T], bf16, tag="hA")
            hB = pp.tile([SU, 1 + T], bf16, tag="hB")
            nc.vector.memset(hA[:, 0:1], 0.0)
            nc.vector.memset(hB[:, 0:1], 0.0)
            hbufs = [hA, hB]

            yp = pp  # y staging tiles live in the persist pool
            y_tiles = [yp.tile([4 * NB, TC], f32, tag=f"y{i}", name=f"y_t{i}")
                       for i in range(2)]

            # ---------- phase-2 pipelined round ----------
            st = {}

            def sweep_round(k, j, hold, hnew, zpsp, final=False):
                if j < NCH:
                    s_t = sp.tile([SU, GATES * TC], bf16, tag="s")
                    if k == 0:
                        src = zpre96[:, :].rearrange(
                            "p (g t) -> p g t", g=GATES)[:, :, j * TC:(j + 1) * TC]
                        dst = s_t[:, :].rearrange("p (g t) -> p g t", g=GATES)
                        nc.scalar.activation(dst, src, AF.Sigmoid)
                    else:
                        zps = zpsp.tile([SU, GATES * TC], f32, tag="zps")
                        for g in range(GATES):
                            nc.tensor.matmul(
                                zps[:, g * TC:(g + 1) * TC], i96_t[:, :],
                                zpre96[:, g * T + j * TC:g * T + (j + 1) * TC],
                                start=True, stop=False)
                        for g in range(GATES):
                            nc.tensor.matmul(
                                zps[:, g * TC:(g + 1) * TC],
                                u4_t[:, SU * g:SU * (g + 1)],
                                hold[:, j * TC:(j + 1) * TC],
                                start=False, stop=True)
                        nc.scalar.activation(s_t[:, :], zps[:, :], AF.Sigmoid)
                        if final:
                            st[("zps", j)] = zps
                    st[("s", j)] = s_t
                jj = j - 1
                if 0 <= jj:
                    s_t = st.pop(("s", jj))
                    ig = igp.tile([SU, TC], bf16, tag="ig")
                    nc.vector.tensor_tensor(
                        out=ig[:, :], in0=s_t[:, 0:TC],
                        in1=s_t[:, 2 * TC:3 * TC], op=OP.mult)
                    c_t = cpl.tile([SU, TC], bf16, tag="c")
                    init = 0.0 if jj == 0 else st["c_prev"][:, TC - 1:TC]
                    nc.vector.tensor_tensor_scan(
                        out=c_t[:, :], data0=s_t[:, TC:2 * TC], data1=ig[:, :],
                        initial=init, op0=OP.mult, op1=OP.add)
                    st["c_prev"] = c_t
                    sc = scp.tile([SU, TC], bf16, tag="sc")
                    nc.scalar.activation(sc[:, :], c_t[:, :], AF.Sigmoid)
                    nc.vector.tensor_tensor(
                        out=hnew[:, 1 + jj * TC:1 + (jj + 1) * TC],
                        in0=s_t[:, 3 * TC:4 * TC], in1=sc[:, :], op=OP.mult)
                    if final:
                        # dense+sigmoid for chunk jj, reusing the freed gate-0
                        # bank of chunk jj's zps tile as the PSUM target
                        zt = st.pop(("zps", jj))
                        nc.tensor.matmul(
                            zt[0:4 * NB, 0:TC], s3_t[:, :],
                            hnew[:, 1 + jj * TC:1 + (jj + 1) * TC],
                            start=True, stop=True)
                        y_t = y_tiles[jj % 2]
                        nc.scalar.activation(y_t[:, :], zt[0:4 * NB, 0:TC],
                                             AF.Sigmoid, bias=bdv_t[:, :])
                        next_q().dma_start(y_d.ap()[:, jj * TC:(jj + 1) * TC],
                                           y_t[:, :])

            # ---------- phase 1 ----------
            with tc.tile_pool(name="xp", bufs=1) as xp:
                xbig = xp.tile([128, NPAIR * T], bf16, tag="xbig")
                # 4 large DMAs, alternating queues; 24KB contiguous per
                # partition row per transfer
                for i in range(4):
                    seg = NPAIR * T // 4
                    next_q().dma_start(xbig[:, i * seg:(i + 1) * seg],
                                       xt_d.ap()[:, i * seg:(i + 1) * seg])

                def warm(n, wt):
                    # PE warm-keeping: dummy matmuls into a scratch PSUM
                    # region (never read) so HAM doesn't re-throttle.
                    for i in range(n):
                        nc.tensor.matmul(wt[0:32, 0:TC], s1_t[:, :],
                                         xbig[:, (i % 8) * TC:
                                              (i % 8) * TC + TC],
                                         start=True, stop=True)

                with tc.tile_pool(name="ps1", bufs=2, space="PSUM") as ps1p:
                    def compute_half(half):
                        cpy = half * 12
                        for tau in range(3):
                            for jc in range(4):
                                pt = ps1p.tile([128, TC], f32, tag="p1")
                                for p in range(4):
                                    # x layout: [128, (half, pair, HT)]
                                    col = (half * NPAIR * HT
                                           + (4 * tau + p) * HT + jc * TC)
                                    nc.tensor.matmul(
                                        pt[32 * p:32 * p + 32, :], s1_t[:, :],
                                        xbig[:, col:col + TC],
                                        start=True, stop=True,
                                        tile_position=(0, 32 * p))
                                dcol = half * HT + jc * TC
                                dst = zpre3[tau][:, dcol:dcol + TC]
                                if cpy % 2 == 0:
                                    nc.scalar.activation(dst, pt[:, :],
                                                         AF.Identity,
                                                         bias=bias1_t[:, :])
                                else:
                                    nc.vector.tensor_scalar(
                                        dst, pt[:, :], bias1_t[:, :], None,
                                        op0=OP.add)
                                cpy += 1

                    def regroup_half(half):
                        # zpre3 lanes within a 32-block are (m, u, g) =
                        # 16m+4u+g, so a contiguous [32, HT] source streams
                        # elementwise as ((m,u), g, t) -- exactly the dst
                        # [8 parts, 4 g-blocks, HT] iteration order.
                        for tau in range(3):
                            for q in range(4):
                                src = zpre3[tau][32 * q:32 * q + 32,
                                                 half * HT:(half + 1) * HT]
                                dst = zpre96[32 * tau + 8 * q:
                                             32 * tau + 8 * q + 8, :] \
                                    .rearrange("p (g t) -> p g t", g=GATES) \
                                    [:, :, half * HT:(half + 1) * HT]
                                next_q().dma_start(dst, src)

                    compute_half(0)
                    regroup_half(0)
                    # sweep-1 first half overlaps the half-1 input DMA
                    for j in range(NCH // 2):
                        sweep_round(0, j, hbufs[0], hbufs[1], None)
                        wt = ps1p.tile([128, TC], f32, tag="wup")
                        warm(8, wt)
                    compute_half(1)
                    regroup_half(1)

                with tc.tile_pool(name="zps", bufs=2, space="PSUM") as zpsp:
                    for j in range(NCH // 2, NCH + 1):
                        sweep_round(0, j, hbufs[0], hbufs[1], zpsp)
                        wt = zpsp.tile([SU, GATES * TC], f32, tag="zps")
                        warm(8, wt)
                    for k in range(1, k_iters):
                        hold = hbufs[k % 2]
                        hnew = hbufs[(k + 1) % 2]
                        for j in range(NCH + 1):
                            sweep_round(k, j, hold, hnew, zpsp,
                                        final=(k == k_iters - 1))

    nc.compile()
    return nc


def _host_consts(W, U, b, Wd, bd):
    W = np.asarray(W, np.float32)
    U = np.asarray(U, np.float32)
    b = np.asarray(b, np.float32)
    Wd = np.asarray(Wd, np.float32)
    bd = np.asarray(bd, np.float32)
    bf = ml_dtypes.bfloat16

    # phase-1 psum within-block col order is (m, u, g) = 16m + 4u + g
    s1 = np.zeros((128, 32), np.float32)
    for m in range(2):
        for g in range(GATES):
            for u in range(UNITS):
                s1[64 * m:64 * m + 64, 16 * m + 4 * u + g] = W[:, 3 * g + u]

    # bias per natural lane: lane 32q + 16m + 4u + g -> b[3g+u]
    blk = np.zeros(32, np.float32)
    for g in range(GATES):
        for m in range(2):
            for u in range(UNITS):
                blk[16 * m + 4 * u + g] = b[3 * g + u]
    bias1 = np.tile(blk, 4).reshape(128, 1).astype(np.float32)

    i96 = np.eye(SU, dtype=np.float32)

    u4 = np.zeros((SU, GATES * SU), np.float32)
    for s in range(NS):
        for up in range(UNITS):
            for g in range(GATES):
                for u in range(UNITS):
                    u4[4 * s + up, SU * g + 4 * s + u] = U[up, 3 * g + u]

    s3 = np.zeros((SU, 4 * NB), np.float32)
    for b_ in range(NB):
        for c in range(3):
            s = 3 * b_ + c
            for u in range(UNITS):
                for dd in range(4):
                    s3[4 * s + u, 4 * b_ + dd] = Wd[3 * c + u, dd]
    bdv = np.tile(bd, NB).reshape(4 * NB, 1).astype(np.float32)

    return {"s1": s1.astype(bf), "bias1": bias1, "i96": i96.astype(bf),
            "u4": u4.astype(bf), "s3": s3.astype(bf), "bdv": bdv}


def _host_xt(inputs, T):
    """[B, T, 192] -> per-core [128, (half, pair, HT)] bf16, s = 3*b_local+c."""
    B = inputs.shape[0]
    x = np.asarray(inputs, np.float32).reshape(B, T, 3, F)
    x = np.transpose(x, (0, 2, 3, 1))              # [B, c, F, T]
    x = np.ascontiguousarray(x).astype(ml_dtypes.bfloat16)
    per_core = []
    for k in range(N_CORES):
        xc = x[k * NB:(k + 1) * NB].reshape(NS, F, T)   # s = 3b+c
        xp = xc.reshape(NPAIR, 128, 2, HT)              # rows 64m+f
        xp = np.transpose(xp, (1, 2, 0, 3))             # [128, half, pair, HT]
        per_core.append(np.ascontiguousarray(xp).reshape(128, NPAIR * T))
    return per_core


def kernel(inputs, W, U, b, Wd, bd):
    from concourse.bass_utils import run_bass_kernel_spmd

    B, T, F3 = inputs.shape
    assert (B, T, F3) == (B_FULL, T_FULL, 192)

    key = (T, K_ITERS)
    if key not in _cache:
        _cache[key] = _build_module(T, K_ITERS, debug=False)
    nc = _cache[key]

    consts = _host_consts(W, U, b, Wd, bd)
    xts = _host_xt(inputs, T)
    in_maps = [dict(consts, xt=xts[k]) for k in range(N_CORES)]

    global _last_exec_ns
    res = run_bass_kernel_spmd(nc, in_maps, list(range(N_CORES)), trace=TRACE)
    if res.exec_time_ns is not None:
        _last_exec_ns = res.exec_time_ns
    ys = [res.results[k]["y"] for k in range(N_CORES)]  # [32, T] each

    out = np.empty((B, T, 4), np.float32)
    for k in range(N_CORES):
        blk = ys[k].reshape(NB, 4, T)          # [b, d, t]
        out[k * NB:(k + 1) * NB] = np.transpose(blk, (0, 2, 1))
    return out
